# revision 29
# baseline (speedup 1.0000x reference)
"""GraphTransformerLayer on 8 Trainium2 NeuronCores (Bass/Tile).

Sharding: 8-way along the query-node axis. Each core owns NQ=512 query rows,
computes the full K/V projections (replicated), its slice of the masked
attention, and its slice of the FFN. No collectives; the host concatenates
the 8 output slices.

fp8 DoubleRow (2 k-slices per partition, 2x PE MACs per row) is used where
the per-instruction contraction reaches 256 AND the quantization noise is
affordable: Q/K/V projections (c=512), ctx=attn@V (c=4096 over m), Wo
(c=512), FFN1 (c=512). FFN2 stays bf16 (fp8 there costs ~6e-3 extra error
for zero measured speedup - the tail is latency-bound, not PE-bound).
Scores stay fp8 non-DR (c=64 per head cannot use DoubleRow).

Attention dataflow per core, head h, m-step s (256 nodes = 2 m-tiles):
  sp[m, (j n)]  = k_h[m]*q_h[n]           PE fp8, 2 matmuls [64c,128,512]
  at            = exp(0.125 * sp)         ACT, PSUM->SBUF bf16 [128,1024]
  at8           = at * maskT              DVE bf16*bf16 -> fp8 [128,(2,512)]
  ctx_ps[66]   += [v8|1|pad] DR@ at8      PE fp8 DoubleRow [128x2c, 66, 512]
ctx row 64 is the softmax denominator; after 16 steps per head:
  rec = approx(1/rowsum); bc = gpsimd bcast; ctxT8 = ctx[0:64]*bc (fp8)
Odd heads DMA-shift their ctxT8 slice to partitions 64:127 (compute engines
cannot cross partitions; DMA can). The v bias is dropped from the V
projection and Wo@bv + bo is folded into the residual host-side.

Schedule: one flat 64-iteration stream (4 head-pairs x 16 m-steps). The ctx
matmul for step s issues after the scores of step s+2 (lag 2), so the PE
never queues behind the exp->mask chain, including across head-pair
boundaries. K-projection chunks and V tiles are drained 2 per iteration
inside the loop (ACT takes kT writes, DVE the v8 copies) - the attention
steady state is near 3-way balanced (PE ~“90%, ACT ~75%, DVE ~80%), so
projection work rides the leftover slack and the PE holds its DVFS state.
Wall time is dominated by a hardware utilization throttle (~0.65 average
rate limit when all engines run hot); isolated-PE microbenchmarks run
~1.5-2x faster than the same matmuls inside the full kernel.

Engine-level lore (measured on hw): ACT fp8 writes only work on contiguous
runs; DVE handles strided fp8 fine but fp8 outputs drop it from 2x to 1x
mode; memset cannot encode fp8 constants (ones come from a host tensor);
reciprocal_approx_fast needs SBUF input; GpSimd cannot touch PSUM and its
tensor_tensor is ~2.3x slower than DVE; dual-fp8 ldweights need even sizes/
offsets; matmuls with c<=64 stream ~2x faster per row than c=128 ones;
many small DMAs at startup serialize at ~2.5us of queue latency each, so
constants ride two dtype-bundled transfers.
"""

import sys

if "/opt/trn_rl_repo" not in sys.path:
    sys.path.insert(0, "/opt/trn_rl_repo")

import numpy as np
import ml_dtypes

import concourse.bacc as bacc
import concourse.tile as tile
import concourse.mybir as mybir
from concourse.bass_utils import run_bass_kernel_spmd

BF16 = ml_dtypes.bfloat16
F8NP = ml_dtypes.float8_e4m3
F32 = mybir.dt.float32
BF = mybir.dt.bfloat16
F8 = mybir.dt.float8e4

N = 4096
D = 512
H = 8
DK = 64
DFF = 2048
NCORES = 8
NQ = N // NCORES  # 512 query rows per core
P = 128
EPS = 1e-5
NSTEP = 16   # m-steps of 256 nodes
VS = DK + 2  # v8 slot: 64 dims + ones + pad (even size for fp8 ldweights)
PRE_V = 16   # all v tiles are produced before attention

ALU = mybir.AluOpType
AF = mybir.ActivationFunctionType
DR = mybir.MatmulPerfMode.DoubleRow

# set by test.py to capture a profile
TRACE = False
TRACE_DIR = None
LAST_EXEC_NS = None

_CACHED = None


def _build():
    nc = bacc.Bacc("TRN2", target_bir_lowering=False, debug=False,
                   num_devices=NCORES)

    # ---- DRAM I/O ----
    # fp8 tensors are host-prefolded into DoubleRow layout:
    #   x8[p, (g, j, cols)] = X[g*256 + j*128 + p, cols]
    h8a = nc.dram_tensor("h8a", [P, 2 * N], F8, kind="ExternalInput").ap()
    h8b = nc.dram_tensor("h8b", [P, 2 * N], F8, kind="ExternalInput").ap()
    hq8 = nc.dram_tensor("hq8", [P, 4 * NQ], F8, kind="ExternalInput").ap()
    maskT = nc.dram_tensor("maskT", [N, NQ], BF, kind="ExternalInput").ap()
    wq8 = nc.dram_tensor("wq8", [P, 4 * D], F8, kind="ExternalInput").ap()
    wk8 = nc.dram_tensor("wk8", [P, 4 * D], F8, kind="ExternalInput").ap()
    wv8 = nc.dram_tensor("wv8", [P, 4 * D], F8, kind="ExternalInput").ap()
    w18 = nc.dram_tensor("w18", [P, 4 * DFF], F8, kind="ExternalInput").ap()
    # w2p[p, (ft, e)] = W2.T[ft*128 + p, e]
    w2p = nc.dram_tensor("w2p", [P, 16 * D], BF, kind="ExternalInput").ap()
    # consts are bundled per dtype: one DMA each (the sync DMA queue costs
    # ~2.5us of latency per entry at startup, so many small DMAs serialize)
    fbund = nc.dram_tensor("fbund", [P, 4120], F32, kind="ExternalInput").ap()
    f8bund = nc.dram_tensor("f8bund", [P, 2080], F8, kind="ExternalInput").ap()
    b22 = nc.dram_tensor("b22", [1, D], BF, kind="ExternalInput").ap()
    identb = nc.dram_tensor("identb", [P, P], BF, kind="ExternalInput").ap()
    out = nc.dram_tensor("out", [NQ, D], F32, kind="ExternalOutput").ap()

    with tile.TileContext(nc) as tc:
        _emit(nc, tc, locals())
    nc.compile()
    return nc


def _emit(nc, tc, t):
    h8a, h8b = t["h8a"], t["h8b"]
    hq8, maskT = t["hq8"], t["maskT"]
    wq8, wk8, wv8 = t["wq8"], t["wk8"], t["wv8"]
    w18, w2p = t["w18"], t["w2p"]
    fbund, f8bund, b22 = t["fbund"], t["f8bund"], t["b22"]
    identb, out = t["identb"], t["out"]

    from contextlib import ExitStack

    es = ExitStack()
    with es:
        cpool = es.enter_context(tc.tile_pool(name="const", bufs=1))
        h1pool = es.enter_context(tc.tile_pool(name="h1p", bufs=1))
        qkv_es = ExitStack()
        mpool = qkv_es.enter_context(tc.tile_pool(name="maskp", bufs=1))
        qkvpool = qkv_es.enter_context(tc.tile_pool(name="qkvp", bufs=1))

        # ---- constants: two bundle tiles, sliced into views ----
        fb_sb = cpool.tile([P, 4120], F32, tag="fbund")
        f8b_sb = cpool.tile([P, 2080], F8, tag="f8bund")
        bq_sb = fb_sb[:, 0:4]
        bk_sb = fb_sb[:, 4:8]
        b1_sb = fb_sb[:, 8:24]
        ln_sb = {}
        for k, nm in enumerate(("g1", "b1l", "g2", "b2l")):
            ln_sb[nm] = fb_sb[:, 24 + k * D:24 + (k + 1) * D]
        hq_sb = [fb_sb[:, 2072 + qt * D:2072 + (qt + 1) * D]
                 for qt in range(4)]
        wo8_sb = f8b_sb[:, 0:2048]
        ones8_sb = f8b_sb[:, 2048:2080]
        b2_sb = cpool.tile([1, D], BF, tag="b2")
        identb_sb = cpool.tile([P, P], BF, tag="idb")
        ones_sb = cpool.tile([1, P], BF, tag="ones")
        nc.vector.memset(ones_sb[:], 1.0)
        eps_sb = cpool.tile([P, 1], F32, tag="eps")
        nc.vector.memset(eps_sb[:], EPS)

        # ---- persistent qkv outputs ----
        kT_sb = [qkvpool.tile([P, N], F8, tag=f"kt{i}", name=f"kT{i}")
                 for i in range(4)]
        qT_sb = [qkvpool.tile([P, NQ], F8, tag=f"qt{i}", name=f"qT{i}")
                 for i in range(4)]
        # v8[s]: [128, (j, h, x)] fp8; x: 0..63 v dims, 64 ones, 65 pad
        v8_sb = [qkvpool.tile([P, 2 * H * VS], F8, tag=f"v{i}",
                              name=f"v8_{i}") for i in range(NSTEP)]

        ppool = qkv_es.enter_context(tc.tile_pool(name="projp", bufs=1))
        wq_sb = ppool.tile([P, 4 * D], F8, tag="wq8")
        nc.sync.dma_start(wq_sb[:], wq8[:])
        hq8_sb = ppool.tile([P, 4 * NQ], F8, tag="hq8")
        nc.sync.dma_start(hq8_sb[:], hq8[:])
        nc.sync.dma_start(fb_sb[:], fbund[:])
        nc.sync.dma_start(f8b_sb[:], f8bund[:])
        nc.sync.dma_start(b2_sb[:], b22[:])
        nc.sync.dma_start(identb_sb[:], identb[:])
        wk_sb = ppool.tile([P, 4 * D], F8, tag="wk8")
        nc.sync.dma_start(wk_sb[:], wk8[:])
        wv_sb = ppool.tile([P, 4 * D], F8, tag="wv8")
        nc.sync.dma_start(wv_sb[:], wv8[:])
        h8a_sb = ppool.tile([P, 2 * N], F8, tag="h8a")
        h8b_sb = ppool.tile([P, 2 * N], F8, tag="h8b")
        nc.sync.dma_start(h8a_sb[:], h8a[:])
        # mask first on the gpsimd queue (mask-mul s=0 needs it early);
        # h8b after (first consumer is v8[8] at ~35us)
        mask_sb = mpool.tile([P, 32 * NQ], BF, tag="mask")
        mview = mask_sb.rearrange("p (mt j) -> p mt j", j=NQ)
        mdram = maskT.rearrange("(mt p) j -> p mt j", p=P)
        nc.gpsimd.dma_start(mview[:, 0:4], mdram[:, 0:4])
        nc.gpsimd.dma_start(mview[:, 4:32], mdram[:, 4:32])
        nc.gpsimd.dma_start(h8b_sb[:], h8b[:])



        h8va = h8a_sb.rearrange("p (g j n) -> p g j n", g=2, j=2)
        h8vb = h8b_sb.rearrange("p (g j n) -> p g j n", g=2, j=2)

        def h8v(col):
            return h8va if col < N // 2 else h8vb

        hq8v = hq8_sb.rearrange("p (g j n) -> p g j n", g=2, j=2)
        wqv = wq_sb.rearrange("p (g j o) -> p g j o", g=2, j=2)
        wkv = wk_sb.rearrange("p (g j o) -> p g j o", g=2, j=2)
        wvv = wv_sb.rearrange("p (g j o) -> p g j o", g=2, j=2)

        psa_es = ExitStack()
        psa = psa_es.enter_context(
            tc.tile_pool(name="psatt", bufs=1, space="PSUM"))

        def ps_half():
            return psa.tile([P, 1024], F32, tag="ps", bufs=3,
                            name="sc_ps")[:, 0:512]

        def emit_qT(tt):
            ps = ps_half()
            for g in range(2):
                nc.tensor.matmul(ps[:], wqv[:, g, :, tt * P:(tt + 1) * P],
                                 hq8v[:, g], start=(g == 0), stop=(g == 1),
                                 perf_mode=DR)
            nc.vector.tensor_scalar_add(qT_sb[tt][:], ps[:],
                                        bq_sb[:, tt:tt + 1])

        def emit_kT(tt, ch, eng):
            c0 = ch * 512
            ps = ps_half()
            for g in range(2):
                nc.tensor.matmul(ps[:], wkv[:, g, :, tt * P:(tt + 1) * P],
                                 h8v(c0)[:, g, :, c0 % 2048:c0 % 2048 + 512],
                                 start=(g == 0), stop=(g == 1), perf_mode=DR)
            if eng == "act":
                nc.scalar.activation(kT_sb[tt][:, c0:c0 + 512], ps[:],
                                     AF.Identity, bias=bk_sb[:, tt:tt + 1])
            else:
                nc.vector.tensor_scalar_add(kT_sb[tt][:, c0:c0 + 512], ps[:],
                                            bk_sb[:, tt:tt + 1])

        def emit_v8(s, eng="dve"):
            v8v = v8_sb[s].rearrange("p (j h x) -> p j h x", j=2, x=VS)
            for j in range(2):
                m0 = s * 256 + j * P
                ps = ps_half()
                for g in range(2):
                    nc.tensor.matmul(ps[:], h8v(m0)[:, g, :, m0 % 2048:m0 % 2048 + P],
                                     wvv[:, g], start=(g == 0),
                                     stop=(g == 1), perf_mode=DR)
                nc.vector.tensor_copy(v8v[:, j, :, 0:DK],
                                      ps.rearrange("p (h x) -> p h x", x=DK))
            nc.vector.tensor_copy(
                v8v[:, :, :, DK:DK + 2],
                ones8_sb.rearrange("p (j h x) -> p j h x", j=2, x=2))

        # ---- pre-attention: qT, kT[0][ch0], v8[0:2] ----
        for tt in range(4):
            emit_qT(tt)
        emit_kT(0, 0, "dve")
        for s in range(2):
            emit_v8(s)

        # deferred projection work, drained 2 items/iter inside the attention
        # loop: v8[s] must exist ~2 iters before ctx consumes it, kT[tt] by
        # the start of head-pair tt. ACT takes the kT writes (it has slack),
        # DVE keeps the v8 copies.
        proj_items = []
        vq = list(range(2, NSTEP))
        kq = [(0, ch) for ch in range(1, 8)]
        kq += [(tt, ch) for tt in range(1, 4) for ch in range(8)]
        while vq or kq:
            if kq:
                proj_items.append(("kT",) + kq.pop(0))
            if vq:
                proj_items.append(("v8", vq.pop(0)))
        proj_pos = [0]

        def drain_proj(n):
            for _ in range(n):
                if proj_pos[0] >= len(proj_items):
                    return
                it = proj_items[proj_pos[0]]
                proj_pos[0] += 1
                if it[0] == "v8":
                    emit_v8(it[1])
                else:
                    emit_kT(it[1], it[2], "act")

        # ================= attention =================
        with tc.tile_pool(name="attp", bufs=1) as apool:
            # ctxT8[s]: [128, (j, n)] fp8; head h -> tile h//4, part (h%2)*64,
            # j = (h//2)%2
            ctxT8_sb = [apool.tile([P, 2 * NQ], F8, tag=f"cx{i}",
                                   name=f"ctxT8_{i}") for i in range(2)]
            at8_q = {}
            ctx_ps_by_hp = {}

            def scores_exp_mask(hp, s):
                for i in range(2):
                    po = i * DK
                    sp = psa.tile([P, 1024], F32, tag="ps", bufs=3,
                                  name="sc_ps")
                    for j in range(2):
                        mt = 2 * s + j
                        nc.tensor.matmul(
                            sp[:, j * NQ:(j + 1) * NQ],
                            kT_sb[hp][po:po + DK, mt * P:(mt + 1) * P],
                            qT_sb[hp][po:po + DK, :],
                            start=True, stop=True)
                    at = apool.tile([P, 1024], BF, tag="at", bufs=3,
                                    name="at")
                    nc.scalar.activation(at[:], sp[:], AF.Exp, scale=0.125)
                    at8 = apool.tile([P, 1024], F8, tag="at8", bufs=6,
                                     name="at8")
                    nc.vector.tensor_mul(
                        at8[:], at[:], mask_sb[:, s * 1024:(s + 1) * 1024])
                    at8_q[(hp, s, i)] = at8

            def ctx_step(hp, s):
                if hp not in ctx_ps_by_hp:
                    ctx_ps_by_hp[hp] = [psa.tile([P, NQ], F32, tag="pc",
                                                 bufs=2, name="ctx_ps")
                                        for _ in range(2)]
                ctx_ps = ctx_ps_by_hp[hp]
                for i, h in ((0, 2 * hp), (1, 2 * hp + 1)):
                    v8v = v8_sb[s].rearrange("p (j h x) -> p j h x",
                                             j=2, x=VS)
                    at8v = at8_q.pop((hp, s, i)).rearrange(
                        "p (j n) -> p j n", j=2)
                    nc.tensor.matmul(
                        ctx_ps[i][0:DK + 2, :], v8v[:, :, h, :],
                        at8v[:], start=(s == 0), stop=(s == NSTEP - 1),
                        perf_mode=DR)

            def normalize(hp):
                ctx_ps = ctx_ps_by_hp[hp]
                for i in range(2):
                    h = 2 * hp + i
                    st, j = h // 4, (h // 2) % 2
                    rsum = apool.tile([1, NQ], F32, tag="rsum", bufs=2,
                                      name="rsum")
                    nc.vector.tensor_copy(rsum[:], ctx_ps[i][DK:DK + 1, :])
                    rec = apool.tile([1, NQ], F32, tag="rec", bufs=2,
                                     name="rec")
                    nc.vector.reciprocal_approx_fast(rec[:], rsum[:])
                    bc = apool.tile([P, NQ], F32, tag="bc", bufs=2,
                                    name="bc")
                    nc.gpsimd.partition_broadcast(bc[0:DK, :], rec[:])
                    dstv = ctxT8_sb[st].rearrange("p (j n) -> p j n", j=2)
                    if h % 2 == 0:
                        nc.vector.tensor_mul(dstv[0:DK, j, :],
                                             ctx_ps[i][0:DK, :], bc[0:DK, :])
                    else:
                        tmp8 = apool.tile([P, NQ], F8, tag="tmp8", bufs=1,
                                          name="tmp8")
                        nc.vector.tensor_mul(tmp8[0:DK, :],
                                             ctx_ps[i][0:DK, :], bc[0:DK, :])
                        nc.sync.dma_start(dstv[DK:P, j, :], tmp8[0:DK, :])

            # flat 64-iteration stream; ctx lags scores by 2 so the PE never
            # waits on the exp->mask chain, including across hp boundaries
            TOT = 4 * NSTEP
            for g in range(TOT):
                hp, s = divmod(g, NSTEP)
                scores_exp_mask(hp, s)
                drain_proj(2)
                if g >= 2:
                    hp2, s2 = divmod(g - 2, NSTEP)
                    ctx_step(hp2, s2)
                if s == 1 and hp > 0:
                    normalize(hp - 1)
            ctx_step(3, NSTEP - 2)
            ctx_step(3, NSTEP - 1)
            normalize(3)

            psa_es.close()

            # ---- Wo + residual + LN1 + transpose ----
            h1_sb = [h1pool.tile([P, D], F32, tag=f"h1_{i}", name=f"h1_{i}")
                     for i in range(4)]
            # h1T8[g]: [128, (j, q)] fp8, channel c = g*256 + j*128 + p
            h1T_sb = [h1pool.tile([P, 2 * NQ], F8, tag=f"h1T{i}",
                                  name=f"h1T8_{i}") for i in range(2)]
            wo8v = wo8_sb.rearrange("p (s j e) -> p s j e", s=2, j=2)
            with tc.tile_pool(name="pspost", bufs=4, space="PSUM") as psw, \
                 tc.tile_pool(name="pstp", bufs=2, space="PSUM") as ptp:
                ps_l = []
                for qt in range(4):
                    ps = psw.tile([P, D], F32, tag="po", name="wo_ps")
                    for s in range(2):
                        ctxv = ctxT8_sb[s].rearrange("p (j n) -> p j n", j=2)
                        nc.tensor.matmul(ps[:],
                                         ctxv[:, :, qt * P:(qt + 1) * P],
                                         wo8v[:, s], start=(s == 0),
                                         stop=(s == 1), perf_mode=DR)
                    ps_l.append(ps)
                ident_sb = h1pool.tile([P, P], F32, tag="idf", bufs=1,
                                       name="identf")
                nc.vector.tensor_copy(ident_sb[:], identb_sb[:])
                _layer_norm_multi(nc, h1pool, 0, ps_l[0:2], hq_sb[0:2],
                                  ln_sb["g1"], ln_sb["b1l"], h1_sb[0:2],
                                  eps_sb)
                _layer_norm_multi(nc, h1pool, 2, ps_l[2:4], hq_sb[2:4],
                                  ln_sb["g1"], ln_sb["b1l"], h1_sb[2:4],
                                  eps_sb)
                for qt in range(4):
                    for ct in range(4):
                        tp = ptp.tile([P, P], F32, tag="tp", name="tp")
                        nc.tensor.transpose(tp[:],
                                            h1_sb[qt][:, ct * P:(ct + 1) * P],
                                            ident_sb[:])
                        h1T8v = h1T_sb[ct // 2].rearrange(
                            "p (j q) -> p j q", j=2)
                        nc.vector.tensor_copy(
                            h1T8v[:, ct % 2, qt * P:(qt + 1) * P], tp[:])

        qkv_es.close()

        # ================= FFN (bf16) =================
        with tc.tile_pool(name="ffnp", bufs=1) as fpool, \
             tc.tile_pool(name="psffn", bufs=4, space="PSUM") as psf:
            w1_sb = fpool.tile([P, 4 * DFF], F8, tag="w18")
            nc.sync.dma_start(w1_sb[:], w18[:])
            w1v = w1_sb.rearrange("p (g j f) -> p g j f", g=2, j=2)
            w2big = fpool.tile([P, 16 * D], BF, tag="w2p")
            nc.sync.dma_start(w2big[:], w2p[:])
            w2_sb = [w2big[:, ft * D:(ft + 1) * D] for ft in range(16)]
            fT_sb = [fpool.tile([P, NQ], BF, tag=f"fT{i}", name=f"fT{i}")
                     for i in range(16)]
            for ft in range(16):
                ps = psf.tile([P, NQ], F32, tag="pf", name="f_ps")
                for g in range(2):
                    h1T8v = h1T_sb[g].rearrange("p (j q) -> p j q", j=2)
                    nc.tensor.matmul(ps[:], w1v[:, g, :, ft * P:(ft + 1) * P],
                                     h1T8v[:], start=(g == 0), stop=(g == 1),
                                     perf_mode=DR)
                nc.scalar.activation(fT_sb[ft][:], ps[:], AF.Relu,
                                     bias=b1_sb[:, ft:ft + 1])
            ps_l = []
            for qt in range(4):
                ps = psf.tile([P, D], F32, tag="pf2", name="ff_ps")
                for ft in range(16):
                    nc.tensor.matmul(ps[:], fT_sb[ft][:, qt * P:(qt + 1) * P],
                                     w2_sb[ft][:], start=(ft == 0), stop=False)
                nc.tensor.matmul(ps[:], ones_sb[:], b2_sb[:],
                                 start=False, stop=True)
                ps_l.append(ps)
            h2_l = _layer_norm_multi(nc, h1pool, 4, ps_l, h1_sb,
                                     ln_sb["g2"], ln_sb["b2l"], None, eps_sb)
            oq = [nc.sync, nc.scalar, nc.gpsimd, nc.sync]
            for qt in range(4):
                oq[qt].dma_start(out[qt * P:(qt + 1) * P, :], h2_l[qt][:])


def _layer_norm_multi(nc, pool, uid, ps_l, res_l, g_sb, b_sb, out_l, eps_sb):
    """LN over a batch of n [P, D] tiles. Per-row scalars for all n tiles are
    batched into [P, n] tiles so the whole scalar pipeline is ~16 DVE ops
    instead of ~35*n. out = LN(ps + res) * g + b."""
    n = len(ps_l)
    I32 = mybir.dt.int32

    def big(tag):
        return [pool.tile([P, D], F32, tag=f"{tag}{uid}_{k}", bufs=1,
                          name=f"{tag}{uid}_{k}") for k in range(n)]

    def cols(tag):
        return pool.tile([P, n], F32, tag=f"{tag}{uid}", bufs=1,
                         name=f"{tag}{uid}")

    x = big("lx")
    s1 = cols("ls1")
    for k in range(n):
        nc.vector.scalar_tensor_tensor(x[k][:], ps_l[k][:], 0.0, res_l[k][:],
                                       op0=ALU.add, op1=ALU.add,
                                       accum_out=s1[:, k:k + 1])
    xsq = big("lxq")
    s2 = cols("ls2")
    for k in range(n):
        nc.vector.tensor_mul(xsq[k][:], x[k][:], x[k][:])
    for k in range(n):
        nc.vector.reduce_sum(s2[:, k:k + 1], xsq[k][:],
                             axis=mybir.AxisListType.X)
    nm = cols("lnm")
    nc.vector.tensor_scalar_mul(nm[:], s1[:], -1.0 / D)
    m2 = cols("lm2")
    nc.vector.tensor_mul(m2[:], nm[:], nm[:])
    ve = cols("lve")
    nc.vector.scalar_tensor_tensor(ve[:], s2[:], 1.0 / D, m2[:],
                                   op0=ALU.mult, op1=ALU.subtract)
    nc.vector.tensor_scalar_add(ve[:], ve[:], eps_sb[:])
    # rstd = rsqrt(ve): bit-trick seed + 2 Newton steps (~4e-6 rel)
    rstd = cols("lrs")
    nc.vector.tensor_single_scalar(rstd[:].bitcast(I32), ve[:].bitcast(I32),
                                   1, op=ALU.arith_shift_right)
    nc.vector.tensor_single_scalar(rstd[:].bitcast(I32), rstd[:].bitcast(I32),
                                   0x5F3759DF, op=ALU.subtract)
    nc.vector.tensor_single_scalar(rstd[:].bitcast(I32), rstd[:].bitcast(I32),
                                   -1, op=ALU.mult)
    tq = cols("ltq")
    for _ in range(2):
        nc.vector.tensor_mul(tq[:], rstd[:], rstd[:])
        nc.vector.tensor_mul(tq[:], tq[:], ve[:])
        nc.vector.tensor_scalar_mul(tq[:], tq[:], -0.5)
        nc.vector.tensor_scalar_add(tq[:], tq[:], 1.5)
        nc.vector.tensor_mul(rstd[:], rstd[:], tq[:])
    # xn = (x - mean) * rstd, then gamma/beta
    if out_l is None:
        out_l = xsq  # xsq is dead after the reduce; reuse as output
    for k in range(n):
        nc.vector.tensor_scalar_add(x[k][:], x[k][:], nm[:, k:k + 1])
    for k in range(n):
        nc.vector.tensor_scalar_mul(x[k][:], x[k][:], rstd[:, k:k + 1])
    for k in range(n):
        nc.vector.tensor_mul(out_l[k][:], x[k][:], g_sb[:])
    for k in range(n):
        nc.vector.tensor_add(out_l[k][:], out_l[k][:], b_sb[:])
    return out_l


def _fold_dr(x, ngroups):
    """[C, cols] -> [128, ngroups*2*cols]: out[p, (g, j, c)] = x[g*256+j*128+p, c]."""
    C, cols = x.shape
    assert C == ngroups * 256
    y = x.reshape(ngroups, 2, P, cols).transpose(2, 0, 1, 3)
    return np.ascontiguousarray(y.reshape(P, ngroups * 2 * cols))


def _prep_inputs(inputs):
    h = np.asarray(inputs["h"], np.float32)
    adj = np.asarray(inputs["adj"])
    f32 = np.float32

    def bf(x):
        return np.ascontiguousarray(np.asarray(x, np.float32).astype(BF16))

    hT = np.ascontiguousarray(h.T)  # [D, N]
    adjb = (adj != 0)
    np.fill_diagonal(adjb, True)
    adjb_bf = adjb.astype(BF16)

    wq, wk, wv, wo = (np.asarray(inputs[k], f32)
                      for k in ("Wq", "Wk", "Wv", "Wo"))
    w1, w2 = np.asarray(inputs["W1"], f32), np.asarray(inputs["W2"], f32)

    # wo8 fold: channel c = (4s + 2j + p//64)*64 + p%64 at [p, (s, j, :)]
    woT = wo.T  # [c, e]
    wo8 = np.empty((P, 2, 2, D), f32)
    for s in range(2):
        for j in range(2):
            for blk in range(2):  # p//64
                c0 = (4 * s + 2 * j + blk) * 64
                wo8[blk * 64:(blk + 1) * 64, s, j, :] = woT[c0:c0 + 64, :]
    wo8 = np.ascontiguousarray(wo8.reshape(P, 4 * D))

    shared = {
        "h8a": _fold_dr(np.ascontiguousarray(hT[:, 0:N // 2]), 2).astype(F8NP),
        "h8b": _fold_dr(np.ascontiguousarray(hT[:, N // 2:]), 2).astype(F8NP),
        "wq8": _fold_dr(wq.T, 2).astype(F8NP),
        "wk8": _fold_dr(wk.T, 2).astype(F8NP),
        "wv8": _fold_dr(wv.T, 2).astype(F8NP),
        "w18": _fold_dr(w1.T, 2).astype(F8NP),
        "w2T": bf(w2.T),
        "b22": bf(np.asarray(inputs["b2"], f32)[None, :]),
        "identb": np.eye(P, dtype=f32).astype(BF16),
    }
    # w2p fold: [p, (ft, e)]
    shared["w2p"] = np.ascontiguousarray(
        w2.T.astype(BF16).reshape(16, P, D).transpose(1, 0, 2).reshape(
            P, 16 * D))
    # fp8 bundle: wo8 + ones
    f8bund = np.empty((P, 2080), f32)
    f8bund[:, 0:2048] = wo8
    f8bund[:, 2048:2080] = 1.0
    shared["f8bund"] = f8bund.astype(F8NP)
    # f32 bundle prototype (per-core hq rows differ)
    fbund = np.empty((P, 4120), f32)
    fbund[:, 0:4] = np.asarray(inputs["bq"], f32).reshape(4, P).T
    fbund[:, 4:8] = np.asarray(inputs["bk"], f32).reshape(4, P).T
    fbund[:, 8:24] = np.asarray(inputs["b1"], f32).reshape(16, P).T
    for k, nm in enumerate(("ln1_g", "ln1_b", "ln2_g", "ln2_b")):
        fbund[:, 24 + k * D:24 + (k + 1) * D] = np.broadcast_to(
            np.asarray(inputs[nm], f32), (P, D))
    bo = np.asarray(inputs["bo"], f32)
    bv = np.asarray(inputs["bv"], f32)
    # ctx is accumulated without the v bias; fold Wo@bv + bo into the residual
    res_bias = bv @ wo.T + bo
    in_maps = []
    for i in range(NCORES):
        r0 = i * NQ
        m = dict(shared)
        m["hq8"] = _fold_dr(hT[:, r0:r0 + NQ], 2).astype(F8NP)
        fb = fbund.copy()
        hqr = h[r0:r0 + NQ, :] + res_bias
        for qt in range(4):
            fb[:, 2072 + qt * D:2072 + (qt + 1) * D] = \
                hqr[qt * P:(qt + 1) * P, :]
        m["fbund"] = np.ascontiguousarray(fb)
        m["maskT"] = np.ascontiguousarray(adjb_bf[r0:r0 + NQ, :].T)
        in_maps.append(m)
    return in_maps


def kernel(**inputs) -> np.ndarray:
    global _CACHED, LAST_EXEC_NS
    if _CACHED is None:
        _CACHED = _build()
    nc = _CACHED
    in_maps = _prep_inputs(inputs)
    kw = {}
    if TRACE:
        kw = dict(trace=True, tmpdir=TRACE_DIR)
    res = run_bass_kernel_spmd(nc, in_maps, list(range(NCORES)), **kw)
    LAST_EXEC_NS = res.exec_time_ns
    return np.concatenate([res.results[i]["out"] for i in range(NCORES)],
                          axis=0)
